# revision 4
# baseline (speedup 1.0000x reference)
"""Causal self-attention + cross-attention Trainium2 kernel (8 NeuronCores).

Sharding: head-parallel. 16 heads x 2 batches = 32 (b,h) pairs; core c owns
heads {2c, 2c+1} for both batches (its 128 channels of C=1024). Projections
are column-sliced per core; attention runs fully local per head; the output
projection is row-sliced and the 8 partial [B*T, C] outputs (fp16) are summed
on the host (no device collectives).

This version fuses the three phases into one software-pipelined stream:
iteration i projects x-chunk i while running attention for pair i-1 and the
output projection + store for pair i-2's tokens. PSUM is partitioned so the
phases can overlap: score tiles (2x2 banks), projection/output/transpose
tiles (2x1 banks), attention accumulators (2x1 banks). All PSUM evictions go
through the Vector engine (ScalarE is saturated by softmax exp). The softmax
epilogue is fully on-chip: the denominator row (accumulated by a ones-column
in V during the AV matmul) is reciprocal'd on DVE (fp16) and broadcast across
partitions with a rank-1 matmul instead of DMA round-trips.
"""
import sys

sys.path.insert(0, "/opt/trn_rl_repo")

import numpy as np

import concourse.bass as bass
import concourse.tile as tile
from concourse import bacc, mybir
from concourse.bass_utils import run_bass_kernel_spmd

dt = mybir.dt

B, T, TC, C, CC, H, D = 2, 2048, 512, 1024, 512, 16, 64
NCORES = 8
CPC = 128          # channels per core = 2 heads * 64
NT = B * T         # 4096 tokens (batch-major)
NTC = B * TC       # 1024 cross tokens
KT_X = C // 128    # 8 contraction tiles over C
KT_C = CC // 128   # 4 contraction tiles over CC
NCH = NT // 512    # 8 token chunks
NCHC = NTC // 512  # 2 cross token chunks
QC_PER_B = T // 512  # 4 q-chunks per batch
KT_PER_B = T // 128  # 16 k-tiles per batch
LOOKAHEAD = 3      # kt steps issued ahead of their AV in the PE queue


def _build(zero_bias=False):
    f32, f16 = dt.float32, dt.float16
    nc = bacc.Bacc("TRN2", target_bir_lowering=False, debug=False,
                   enable_asserts=True, num_devices=NCORES)

    xT = nc.dram_tensor("xT", [NCH, 128, KT_X, 512], f16, kind="ExternalInput").ap()
    cT = nc.dram_tensor("cT", [NCHC, 128, KT_C, 512], f16, kind="ExternalInput").ap()
    wq = nc.dram_tensor("wq", [128, KT_X, CPC], f16, kind="ExternalInput").ap()
    wk = nc.dram_tensor("wk", [128, KT_X, CPC], f16, kind="ExternalInput").ap()
    wv = nc.dram_tensor("wv", [128, KT_X, CPC], f16, kind="ExternalInput").ap()
    wcq = nc.dram_tensor("wcq", [128, KT_X, CPC], f16, kind="ExternalInput").ap()
    wck = nc.dram_tensor("wck", [128, KT_C, CPC], f16, kind="ExternalInput").ap()
    wcv = nc.dram_tensor("wcv", [128, KT_C, CPC], f16, kind="ExternalInput").ap()
    wp = nc.dram_tensor("wp", [CPC, C], f16, kind="ExternalInput").ap()
    bias6 = nc.dram_tensor("bias6", [CPC, 6], f32, kind="ExternalInput").ap()
    maskd = nc.dram_tensor("mask", [128, 128], f16, kind="ExternalInput").ap()
    identd = nc.dram_tensor("ident", [128, 128], f16, kind="ExternalInput").ap()
    out = nc.dram_tensor("out", [NT, C], f16, kind="ExternalOutput").ap()

    Exp = mybir.ActivationFunctionType.Exp
    SCALE = 0.125  # 1/sqrt(D)

    with tile.TileContext(nc) as tc:
        from contextlib import ExitStack
        with ExitStack() as es:
            persist = es.enter_context(tc.tile_pool(name="persist", bufs=1))
            qT_t = persist.tile([128, NT], f16, tag="qT")
            kT_t = persist.tile([128, NT], f16, tag="kT")
            qcT_t = persist.tile([128, NT], f16, tag="qcT")
            kcT_t = persist.tile([128, NTC], f16, tag="kcT")
            vT_t = persist.tile([128, NT], f16, tag="vT")
            vcT_t = persist.tile([128, NTC], f16, tag="vcT")
            vn_t = persist.tile([128, (NT // 128) * 130], f16, tag="vn")
            vcn_t = persist.tile([128, (NTC // 128) * 130], f16, tag="vcn")
            yT2_t = persist.tile([128, NT], f16, tag="yT2")
            wp_t = persist.tile([128, C], f16, tag="wp")
            bias_t = persist.tile([128, 6], f32, tag="bias")
            mask_t = persist.tile([128, 128], f16, tag="mask")
            ident_t = persist.tile([128, 128], f16, tag="ident")
            ones_t = persist.tile([1, 64], f16, tag="ones")

            wq_t = persist.tile([128, KT_X, CPC], f16, tag="wq")
            wk_t = persist.tile([128, KT_X, CPC], f16, tag="wk")
            wv_t = persist.tile([128, KT_X, CPC], f16, tag="wv")
            wcq_t = persist.tile([128, KT_X, CPC], f16, tag="wcq")
            wck_t = persist.tile([128, KT_C, CPC], f16, tag="wck")
            wcv_t = persist.tile([128, KT_C, CPC], f16, tag="wcv")

            # weight/constant loads; cross weights first (first compute)
            for wdram, wtile in ((wck, wck_t), (wcv, wcv_t), (wq, wq_t),
                                 (wk, wk_t), (wv, wv_t), (wcq, wcq_t)):
                nc.gpsimd.dma_start(out=wtile[:], in_=wdram[:])
            nc.gpsimd.dma_start(out=mask_t[:], in_=maskd[:])
            nc.gpsimd.dma_start(out=ident_t[:], in_=identd[:])
            nc.gpsimd.dma_start(out=bias_t[:], in_=bias6[:])
            nc.gpsimd.dma_start(out=wp_t[:], in_=wp[:])
            nc.vector.memset(ones_t[:], 1.0)

            vn_r = vn_t[:].rearrange("p (t c) -> p t c", c=130)
            nc.vector.memset(vn_r[:, :, 64:65], 1.0)
            nc.vector.memset(vn_r[:, :, 129:130], 1.0)
            vcn_r = vcn_t[:].rearrange("p (t c) -> p t c", c=130)
            nc.vector.memset(vcn_r[:, :, 64:65], 1.0)
            nc.vector.memset(vcn_r[:, :, 129:130], 1.0)

            # working pools
            xpool = es.enter_context(tc.tile_pool(name="xpool", bufs=2))
            cpool = es.enter_context(tc.tile_pool(name="cpool", bufs=2))
            expool = es.enter_context(tc.tile_pool(name="expool", bufs=12))
            ysbpool = es.enter_context(tc.tile_pool(name="ysbpool", bufs=3))
            rpool = es.enter_context(tc.tile_pool(name="rpool", bufs=3))
            yapool = es.enter_context(tc.tile_pool(name="yapool", bufs=6))
            ybspool = es.enter_context(tc.tile_pool(name="ybspool", bufs=2))
            sopool = es.enter_context(tc.tile_pool(name="sopool", bufs=3))
            # PSUM: st 2x2 banks, pj 2x1, yh0/yh1 1+1
            stps = es.enter_context(tc.tile_pool(name="stps", bufs=2, space="PSUM"))
            pjps = es.enter_context(tc.tile_pool(name="pjps", bufs=2, space="PSUM"))
            yh0ps = es.enter_context(tc.tile_pool(name="yh0ps", bufs=1, space="PSUM"))
            yh1ps = es.enter_context(tc.tile_pool(name="yh1ps", bufs=1, space="PSUM"))

            def psum_evict(dst_slice, ps, bcol):
                if zero_bias:
                    nc.vector.tensor_copy(dst_slice, ps[:])
                else:
                    nc.vector.tensor_scalar_add(dst_slice, ps[:],
                                                bias_t[:, bcol:bcol + 1])

            def transpose_v(src, dstn, tt):
                """PE-transpose one 128-token tile of v and scatter both
                heads' halves into the [64|1|64|1] AV layout in one copy."""
                pt = pjps.tile([128, 128], f16, tag="pj")
                nc.tensor.transpose(pt[:], src[:, tt * 128:(tt + 1) * 128],
                                    ident_t[:])
                src3 = pt[:].rearrange("p (b x) -> p b x", b=2)
                dst3 = dstn[:, tt * 130:(tt + 1) * 130].rearrange(
                    "p (b x) -> p b x", b=2, x=65)[:, :, 0:64]
                nc.vector.tensor_copy(dst3, src3)

            def proj_x_chunk(ch, split_load=False):
                xblk = xpool.tile([128, KT_X, 512], f16, tag="xblk")
                if split_load:
                    for kt in range(KT_X):
                        nc.gpsimd.dma_start(out=xblk[:, kt, :], in_=xT[ch, :, kt, :])
                else:
                    nc.gpsimd.dma_start(out=xblk[:], in_=xT[ch])
                for wtile, dst, bcol in ((wq_t, qT_t, 0), (wk_t, kT_t, 1),
                                         (wv_t, vT_t, 2), (wcq_t, qcT_t, 3)):
                    ps = pjps.tile([128, 512], f32, tag="pj")
                    for kt in range(KT_X):
                        nc.tensor.matmul(ps[:], wtile[:, kt, :], xblk[:, kt, :],
                                         start=(kt == 0), stop=(kt == KT_X - 1))
                    psum_evict(dst[:, ch * 512:(ch + 1) * 512], ps, bcol)
                    if dst is vT_t:
                        for tt in range(ch * 4, ch * 4 + 4):
                            transpose_v(vT_t, vn_t, tt)

            def proj_c_chunk(chc):
                cblk = cpool.tile([128, KT_C, 512], f16, tag="cblk")
                nc.gpsimd.dma_start(out=cblk[:], in_=cT[chc])
                for wtile, dst, bcol in ((wck_t, kcT_t, 4), (wcv_t, vcT_t, 5)):
                    ps = pjps.tile([128, 512], f32, tag="pj")
                    for kt in range(KT_C):
                        nc.tensor.matmul(ps[:], wtile[:, kt, :], cblk[:, kt, :],
                                         start=(kt == 0), stop=(kt == KT_C - 1))
                    psum_evict(dst[:, chc * 512:(chc + 1) * 512], ps, bcol)
                    if dst is vcT_t:
                        for tt in range(chc * 4, chc * 4 + 4):
                            transpose_v(vcT_t, vcn_t, tt)

            def attn_part(b, qc, qlo, is_self):
                """One softmax-attention accumulation (self or cross) for a
                512-wide q chunk of batch b. Returns per-head normalized
                [64,512] tiles (ya, yb)."""
                nkt = (4 * qc + 4) if is_self else KT_C
                yh_0 = yh0ps.tile([65, 512], f32, tag="yh0")
                yh_1 = yh1ps.tile([65, 512], f32, tag="yh1")
                yh = (yh_0, yh_1)

                pend = []
                fidx = [0]

                def flush_one():
                    ex, off, vsrc, vc0, vc1 = pend.pop(0)
                    first = fidx[0] == 0
                    last = fidx[0] == nkt - 1
                    fidx[0] += 1
                    nc.tensor.matmul(
                        yh[0][:, off:512],
                        vsrc[:, vc0:vc0 + 65],
                        ex[:, off:512],
                        start=first, stop=last)
                    nc.tensor.matmul(
                        yh[1][:, off:512],
                        vsrc[:, vc1:vc1 + 65],
                        ex[:, 512 + off:1024],
                        start=first, stop=last)

                for kt in range(nkt):
                    if is_self:
                        crossing = kt >= 4 * qc
                        off = (kt - 4 * qc) * 128 if crossing else 0
                        klo = b * T + kt * 128
                        ksrc, qsrc, vsrc = kT_t, qT_t, vn_t
                        vbase = (b * KT_PER_B + kt) * 130
                    else:
                        crossing, off = False, 0
                        klo = b * TC + kt * 128
                        ksrc, qsrc, vsrc = kcT_t, qcT_t, vcn_t
                        vbase = (b * KT_C + kt) * 130
                    st = stps.tile([128, 1024], f32, tag="st")
                    nc.tensor.matmul(
                        st[:, off:512],
                        ksrc[0:64, klo:klo + 128],
                        qsrc[0:64, qlo + off:qlo + 512],
                        start=True, stop=True)
                    nc.tensor.matmul(
                        st[:, 512 + off:1024],
                        ksrc[64:128, klo:klo + 128],
                        qsrc[64:128, qlo + off:qlo + 512],
                        start=True, stop=True)
                    ex = expool.tile([128, 1024], f16, tag="ex")
                    if off == 0:
                        nc.scalar.activation(ex[:], st[:], Exp, scale=SCALE)
                    else:
                        nc.scalar.activation(ex[:, off:512], st[:, off:512],
                                             Exp, scale=SCALE)
                        nc.scalar.activation(ex[:, 512 + off:1024],
                                             st[:, 512 + off:1024],
                                             Exp, scale=SCALE)
                    if crossing:
                        nc.vector.tensor_mul(ex[:, off:off + 128],
                                             ex[:, off:off + 128], mask_t[:])
                        nc.vector.tensor_mul(ex[:, 512 + off:512 + off + 128],
                                             ex[:, 512 + off:512 + off + 128],
                                             mask_t[:])
                    pend.append((ex, off, vsrc, vbase, vbase + 65))
                    if len(pend) > LOOKAHEAD:
                        flush_one()
                while pend:
                    flush_one()

                # evict accumulators to SBUF (releases the two yh banks),
                # then 1/denominator + rank-1 matmul partition-broadcast
                ysb = ysbpool.tile([65, 1024], f32, tag="ysb")
                nc.vector.tensor_copy(ysb[:, 0:512], yh[0][:])
                nc.vector.tensor_copy(ysb[:, 512:1024], yh[1][:])
                rsb = rpool.tile([1, 1024], f16, tag="rsb")
                with nc.allow_low_precision("softmax 1/denom in fp16"):
                    nc.vector.reciprocal(rsb[:], ysb[64:65, :])
                bc0 = yh0ps.tile([64, 512], f32, tag="yh0")
                nc.tensor.matmul(bc0[:], ones_t[:], rsb[0:1, 0:512],
                                 start=True, stop=True)
                bc1 = yh1ps.tile([64, 512], f32, tag="yh1")
                nc.tensor.matmul(bc1[:], ones_t[:], rsb[0:1, 512:1024],
                                 start=True, stop=True)
                ya = yapool.tile([64, 512], f32, tag="ya")
                nc.vector.tensor_mul(ya[:], ysb[0:64, 0:512], bc0[:])
                yb = yapool.tile([64, 512], f32, tag="ya")
                nc.vector.tensor_mul(yb[:], ysb[0:64, 512:1024], bc1[:])
                return ya, yb

            def combine_pair(qlo, part_c, part_s):
                yac, ybc = part_c
                yas, ybs = part_s
                nc.vector.tensor_add(yT2_t[0:64, qlo:qlo + 512], yas[:], yac[:])
                ybsum = ybspool.tile([64, 512], f16, tag="ybsum")
                nc.vector.tensor_add(ybsum[:], ybs[:], ybc[:])
                # partition shift rows 0-63 -> 64-127 via SBUF-SBUF DMA
                nc.gpsimd.dma_start(out=yT2_t[64:128, qlo:qlo + 512],
                                    in_=ybsum[:])

            def phase_c_pair(qlo):
                for tt in range(qlo // 128, qlo // 128 + 4):
                    so = sopool.tile([128, C], f16, tag="so")
                    for co in range(2):
                        po = pjps.tile([128, 512], f32, tag="pj")
                        nc.tensor.matmul(po[:],
                                         yT2_t[:, tt * 128:(tt + 1) * 128],
                                         wp_t[:, co * 512:(co + 1) * 512],
                                         start=True, stop=True)
                        nc.vector.tensor_copy(so[:, co * 512:(co + 1) * 512],
                                              po[:])
                    nc.gpsimd.dma_start(
                        out=out[tt * 128:(tt + 1) * 128, :], in_=so[:])

            # ---------------- fused pipeline ----------------
            proj_c_chunk(0)
            proj_x_chunk(0, split_load=True)
            proj_c_chunk(1)
            for i in range(1, NCH + 1):
                b, qc = divmod(i - 1, QC_PER_B)
                qlo = b * T + qc * 512
                part_c = attn_part(b, qc, qlo, is_self=False)
                part_s = attn_part(b, qc, qlo, is_self=True)
                if i < NCH:
                    proj_x_chunk(i)
                combine_pair(qlo, part_c, part_s)
                phase_c_pair(qlo)

    nc.compile()
    return nc


_NC_CACHE = {}


def _get_nc(zero_bias=False):
    if zero_bias not in _NC_CACHE:
        _NC_CACHE[zero_bias] = _build(zero_bias)
    return _NC_CACHE[zero_bias]


def warr(w):
    """[C,128] weight -> [128, KT, 128] fp16 (partition-major k-tiles)."""
    kt = w.shape[0] // 128
    return np.ascontiguousarray(
        w.reshape(kt, 128, w.shape[1]).transpose(1, 0, 2)).astype(np.float16)


def make_in_maps(x, cross_input, Wk, bk, Wq, bq, Wv, bv, Wck, bck, Wcq, bcq,
                 Wcv, bcv, Wp, bp):
    """Host-side shard + layout prep. Returns per-core input maps."""
    xT0 = np.asarray(x, np.float32).reshape(NT, C).T.astype(np.float16)  # [C, NT]
    xT = np.ascontiguousarray(
        xT0.reshape(KT_X, 128, NCH, 512).transpose(2, 1, 0, 3))  # [NCH,128,KT,512]
    cT0 = np.asarray(cross_input, np.float32).reshape(NTC, CC).T.astype(np.float16)
    cT = np.ascontiguousarray(
        cT0.reshape(KT_C, 128, NCHC, 512).transpose(2, 1, 0, 3))
    mask = np.triu(np.ones((128, 128), np.float32)).astype(np.float16)  # 1 iff kk<=qq
    Wq, Wk, Wv = (np.asarray(w, np.float32) for w in (Wq, Wk, Wv))
    Wcq, Wck, Wcv = (np.asarray(w, np.float32) for w in (Wcq, Wck, Wcv))
    Wp = np.asarray(Wp, np.float32)
    in_maps = []
    for c in range(NCORES):
        sl = slice(c * CPC, (c + 1) * CPC)
        bias6 = np.stack([np.asarray(v, np.float32)[sl] for v in
                          (bq, bk, bv, bcq, bck, bcv)], axis=1)
        in_maps.append({
            "xT": xT, "cT": cT,
            "wq": warr(Wq[:, sl]), "wk": warr(Wk[:, sl]),
            "wv": warr(Wv[:, sl]), "wcq": warr(Wcq[:, sl]),
            "wck": warr(Wck[:, sl]), "wcv": warr(Wcv[:, sl]),
            "wp": Wp[sl, :].astype(np.float16),
            "bias6": np.ascontiguousarray(bias6),
            "mask": mask,
            "ident": np.eye(128, dtype=np.float16),
        })
    return in_maps


def kernel(**inputs):
    in_maps = make_in_maps(**inputs)
    zb = all(not np.any(np.asarray(inputs[k])) for k in
             ("bq", "bk", "bv", "bcq", "bck", "bcv"))
    nc = _get_nc(zero_bias=zb)
    res = run_bass_kernel_spmd(nc, in_maps, list(range(NCORES)))
    acc = np.zeros((NT, C), np.float64)
    for c in range(NCORES):
        acc += res.results[c]["out"]
    acc += np.asarray(inputs["bp"], np.float32)
    return acc.reshape(B, T, C).astype(np.float32)


if __name__ == "__main__":
    nc = _get_nc()
    print("build + compile OK")


# revision 13
# speedup vs baseline: 1.2921x; 1.2921x over previous
"""Causal self-attention + cross-attention Trainium2 kernel (8 NeuronCores).

Sharding: head-parallel. 16 heads x 2 batches = 32 (b,h) pairs; core c owns
heads {2c, 2c+1} for both batches (its 128 channels of C=1024). Projections
are column-sliced per core; attention runs fully local per head; the output
projection is row-sliced and the 8 partial [B*T, C] outputs (fp16) are summed
on the host (no device collectives).

This version fuses the three phases into one software-pipelined stream:
iteration i projects x-chunk i while running attention for pair i-1 and the
output projection + store for pair i-2's tokens. PSUM is partitioned so the
phases can overlap: score tiles (2x2 banks), projection/output/transpose
tiles (2x1 banks), attention accumulators (2x1 banks). All PSUM evictions go
through the Vector engine (ScalarE is saturated by softmax exp). The softmax
epilogue is fully on-chip: the denominator row (accumulated by a ones-column
in V during the AV matmul) is reciprocal'd on DVE (fp16) and broadcast across
partitions with a rank-1 matmul instead of DMA round-trips.
"""
import sys

sys.path.insert(0, "/opt/trn_rl_repo")

import numpy as np

import concourse.bass as bass
import concourse.tile as tile
from concourse import bacc, mybir
from concourse.bass_utils import run_bass_kernel_spmd

dt = mybir.dt

B, T, TC, C, CC, H, D = 2, 2048, 512, 1024, 512, 16, 64
NCORES = 8
CPC = 128          # channels per core = 2 heads * 64
NT = B * T         # 4096 tokens (batch-major)
NTC = B * TC       # 1024 cross tokens
KT_X = C // 128    # 8 contraction tiles over C
KT_C = CC // 128   # 4 contraction tiles over CC
NCH = NT // 512    # 8 token chunks
NCHC = NTC // 512  # 2 cross token chunks
QC_PER_B = T // 512  # 4 q-chunks per batch
KT_PER_B = T // 128  # 16 k-tiles per batch
LOOKAHEAD = 3      # kt steps issued ahead of their AV in the PE queue


def _build(zero_bias=False):
    f32, f16 = dt.float32, dt.float16
    nc = bacc.Bacc("TRN2", target_bir_lowering=False, debug=False,
                   enable_asserts=True, num_devices=NCORES)

    xT = nc.dram_tensor("xT", [NCH, 128, KT_X, 512], f16, kind="ExternalInput").ap()
    cT = nc.dram_tensor("cT", [NCHC, 128, KT_C, 512], f16, kind="ExternalInput").ap()
    wq = nc.dram_tensor("wq", [128, KT_X, CPC], f16, kind="ExternalInput").ap()
    wk = nc.dram_tensor("wk", [128, KT_X, CPC], f16, kind="ExternalInput").ap()
    wv = nc.dram_tensor("wv", [128, KT_X, CPC], f16, kind="ExternalInput").ap()
    wcq = nc.dram_tensor("wcq", [128, KT_X, CPC], f16, kind="ExternalInput").ap()
    wck = nc.dram_tensor("wck", [128, KT_C, CPC], f16, kind="ExternalInput").ap()
    wcv = nc.dram_tensor("wcv", [128, KT_C, CPC], f16, kind="ExternalInput").ap()
    wp = nc.dram_tensor("wp", [CPC, C], f16, kind="ExternalInput").ap()
    bias6 = nc.dram_tensor("bias6", [CPC, 6], f32, kind="ExternalInput").ap()
    maskd = nc.dram_tensor("mask", [128, 128], f16, kind="ExternalInput").ap()
    identd = nc.dram_tensor("ident", [128, 128], f16, kind="ExternalInput").ap()
    out = nc.dram_tensor("out", [NT, C], f16, kind="ExternalOutput").ap()

    Exp = mybir.ActivationFunctionType.Exp
    SCALE = 0.125  # 1/sqrt(D)

    with tile.TileContext(nc) as tc:
        from contextlib import ExitStack
        with ExitStack() as es:
            persist = es.enter_context(tc.tile_pool(name="persist", bufs=1))
            qT_t = persist.tile([128, NT], f16, tag="qT")
            kT_t = persist.tile([128, NT], f16, tag="kT")
            qcT_t = persist.tile([128, NT], f16, tag="qcT")
            kcT_t = persist.tile([128, NTC], f16, tag="kcT")
            vT_t = persist.tile([128, NT], f16, tag="vT")
            vcT_t = persist.tile([128, NTC], f16, tag="vcT")
            vn_t = persist.tile([128, (NT // 128) * 130], f16, tag="vn")
            vcn_t = persist.tile([128, (NTC // 128) * 130], f16, tag="vcn")
            yT2_t = persist.tile([128, NT], f16, tag="yT2")
            wp_t = persist.tile([128, C], f16, tag="wp")
            bias_t = persist.tile([128, 6], f32, tag="bias")
            mask_t = persist.tile([128, 128], f16, tag="mask")
            ident_t = persist.tile([128, 128], f16, tag="ident")
            ones_t = persist.tile([1, 64], f16, tag="ones")

            wq_t = persist.tile([128, KT_X, CPC], f16, tag="wq")
            wk_t = persist.tile([128, KT_X, CPC], f16, tag="wk")
            wv_t = persist.tile([128, KT_X, CPC], f16, tag="wv")
            wcq_t = persist.tile([128, KT_X, CPC], f16, tag="wcq")
            wck_t = persist.tile([128, KT_C, CPC], f16, tag="wck")
            wcv_t = persist.tile([128, KT_C, CPC], f16, tag="wcv")

            # weight/constant loads; cross weights first (first compute),
            # spread across issue queues so dispatch doesn't serialize
            nc.sync.dma_start(out=wck_t[:], in_=wck[:])
            nc.scalar.dma_start(out=wcv_t[:], in_=wcv[:])
            nc.sync.dma_start(out=wq_t[:], in_=wq[:])
            nc.scalar.dma_start(out=wk_t[:], in_=wk[:])
            nc.sync.dma_start(out=wv_t[:], in_=wv[:])
            nc.scalar.dma_start(out=wcq_t[:], in_=wcq[:])
            nc.sync.dma_start(out=mask_t[:], in_=maskd[:])
            nc.scalar.dma_start(out=ident_t[:], in_=identd[:])
            nc.sync.dma_start(out=bias_t[:], in_=bias6[:])
            nc.sync.dma_start(out=wp_t[:], in_=wp[:])
            nc.vector.memset(ones_t[:], 1.0)

            vn_r = vn_t[:].rearrange("p (t c) -> p t c", c=130)
            nc.vector.memset(vn_r[:, :, 64:65], 1.0)
            nc.vector.memset(vn_r[:, :, 129:130], 1.0)
            vcn_r = vcn_t[:].rearrange("p (t c) -> p t c", c=130)
            nc.vector.memset(vcn_r[:, :, 64:65], 1.0)
            nc.vector.memset(vcn_r[:, :, 129:130], 1.0)

            # working pools
            xpool = es.enter_context(tc.tile_pool(name="xpool", bufs=2))
            cpool = es.enter_context(tc.tile_pool(name="cpool", bufs=2))
            expool = es.enter_context(tc.tile_pool(name="expool", bufs=12))
            ysbpool = es.enter_context(tc.tile_pool(name="ysbpool", bufs=3))
            rpool = es.enter_context(tc.tile_pool(name="rpool", bufs=3))
            yapool = es.enter_context(tc.tile_pool(name="yapool", bufs=6))
            ybspool = es.enter_context(tc.tile_pool(name="ybspool", bufs=2))
            sopool = es.enter_context(tc.tile_pool(name="sopool", bufs=3))
            rdpool = es.enter_context(tc.tile_pool(name="rdpool", bufs=2,
                                                   space="DRAM"))
            # PSUM: st 2x2 banks, pj 2x1, yh0/yh1 1+1
            stps = es.enter_context(tc.tile_pool(name="stps", bufs=2, space="PSUM"))
            pjps = es.enter_context(tc.tile_pool(name="pjps", bufs=2, space="PSUM"))
            yh0ps = es.enter_context(tc.tile_pool(name="yh0ps", bufs=1, space="PSUM"))
            yh1ps = es.enter_context(tc.tile_pool(name="yh1ps", bufs=1, space="PSUM"))

            def psum_evict(dst_slice, ps, bcol):
                if zero_bias:
                    nc.vector.tensor_copy(dst_slice, ps[:])
                else:
                    nc.vector.tensor_scalar_add(dst_slice, ps[:],
                                                bias_t[:, bcol:bcol + 1])

            def transpose_v(src, dstn, tt):
                """PE-transpose one 128-token tile of v and scatter both
                heads' halves into the [64|1|64|1] AV layout in one copy."""
                pt = pjps.tile([128, 128], f16, tag="pj")
                nc.tensor.transpose(pt[:], src[:, tt * 128:(tt + 1) * 128],
                                    ident_t[:])
                src3 = pt[:].rearrange("p (b x) -> p b x", b=2)
                dst3 = dstn[:, tt * 130:(tt + 1) * 130].rearrange(
                    "p (b x) -> p b x", b=2, x=65)[:, :, 0:64]
                nc.vector.tensor_copy(dst3, src3)

            def proj_x_chunk(ch, split_load=False):
                xblk = xpool.tile([128, KT_X, 512], f16, tag="xblk")
                if split_load:
                    for kt in range(KT_X):
                        nc.gpsimd.dma_start(out=xblk[:, kt, :], in_=xT[ch, :, kt, :])
                else:
                    nc.gpsimd.dma_start(out=xblk[:], in_=xT[ch])
                for wtile, dst, bcol in ((wq_t, qT_t, 0), (wk_t, kT_t, 1),
                                         (wv_t, vT_t, 2), (wcq_t, qcT_t, 3)):
                    ps = pjps.tile([128, 512], f32, tag="pj")
                    for kt in range(KT_X):
                        nc.tensor.matmul(ps[:], wtile[:, kt, :], xblk[:, kt, :],
                                         start=(kt == 0), stop=(kt == KT_X - 1))
                    psum_evict(dst[:, ch * 512:(ch + 1) * 512], ps, bcol)
                    if dst is vT_t:
                        for tt in range(ch * 4, ch * 4 + 4):
                            transpose_v(vT_t, vn_t, tt)

            def proj_c_chunk(chc):
                cblk = cpool.tile([128, KT_C, 512], f16, tag="cblk")
                nc.gpsimd.dma_start(out=cblk[:], in_=cT[chc])
                for wtile, dst, bcol in ((wck_t, kcT_t, 4), (wcv_t, vcT_t, 5)):
                    ps = pjps.tile([128, 512], f32, tag="pj")
                    for kt in range(KT_C):
                        nc.tensor.matmul(ps[:], wtile[:, kt, :], cblk[:, kt, :],
                                         start=(kt == 0), stop=(kt == KT_C - 1))
                    psum_evict(dst[:, chc * 512:(chc + 1) * 512], ps, bcol)
                    if dst is vcT_t:
                        for tt in range(chc * 4, chc * 4 + 4):
                            transpose_v(vcT_t, vcn_t, tt)

            def attn_part(b, qc, qlo, is_self):
                """One softmax-attention accumulation (self or cross) for a
                512-wide q chunk of batch b. Returns the unnormalized
                [65,1024] SBUF accumulator (row 64 = denominators)."""
                nkt = (4 * qc + 4) if is_self else KT_C
                yh_0 = yh0ps.tile([65, 512], f32, tag="yh0")
                yh_1 = yh1ps.tile([65, 512], f32, tag="yh1")
                yh = (yh_0, yh_1)

                pend = []
                fidx = [0]

                def flush_one():
                    ex, off, vsrc, vc0, vc1 = pend.pop(0)
                    first = fidx[0] == 0
                    last = fidx[0] == nkt - 1
                    fidx[0] += 1
                    nc.tensor.matmul(
                        yh[0][:, off:512],
                        vsrc[:, vc0:vc0 + 65],
                        ex[:, off:512],
                        start=first, stop=last)
                    nc.tensor.matmul(
                        yh[1][:, off:512],
                        vsrc[:, vc1:vc1 + 65],
                        ex[:, 512 + off:1024],
                        start=first, stop=last)

                for kt in range(nkt):
                    if is_self:
                        crossing = kt >= 4 * qc
                        off = (kt - 4 * qc) * 128 if crossing else 0
                        klo = b * T + kt * 128
                        ksrc, qsrc, vsrc = kT_t, qT_t, vn_t
                        vbase = (b * KT_PER_B + kt) * 130
                    else:
                        crossing, off = False, 0
                        klo = b * TC + kt * 128
                        ksrc, qsrc, vsrc = kcT_t, qcT_t, vcn_t
                        vbase = (b * KT_C + kt) * 130
                    st = stps.tile([128, 1024], f32, tag="st")
                    nc.tensor.matmul(
                        st[:, off:512],
                        ksrc[0:64, klo:klo + 128],
                        qsrc[0:64, qlo + off:qlo + 512],
                        start=True, stop=True)
                    nc.tensor.matmul(
                        st[:, 512 + off:1024],
                        ksrc[64:128, klo:klo + 128],
                        qsrc[64:128, qlo + off:qlo + 512],
                        start=True, stop=True)
                    ex = expool.tile([128, 1024], f16, tag="ex")
                    if off == 0:
                        nc.scalar.activation(ex[:], st[:], Exp, scale=SCALE)
                    else:
                        nc.scalar.activation(ex[:, off:512], st[:, off:512],
                                             Exp, scale=SCALE)
                        nc.scalar.activation(ex[:, 512 + off:1024],
                                             st[:, 512 + off:1024],
                                             Exp, scale=SCALE)
                    if crossing:
                        nc.vector.tensor_mul(ex[:, off:off + 128],
                                             ex[:, off:off + 128], mask_t[:])
                        nc.vector.tensor_mul(ex[:, 512 + off:512 + off + 128],
                                             ex[:, 512 + off:512 + off + 128],
                                             mask_t[:])
                    pend.append((ex, off, vsrc, vbase, vbase + 65))
                    if len(pend) > LOOKAHEAD:
                        flush_one()
                while pend:
                    flush_one()

                # evict accumulators to SBUF (releases the two yh banks)
                ysb = ysbpool.tile([65, 1024], f32, tag="ysb")
                nc.vector.tensor_copy(ysb[:, 0:512], yh[0][:])
                nc.vector.tensor_copy(ysb[:, 512:1024], yh[1][:])
                return ysb

            def combine_pair(qlo, ysb_c, ysb_s):
                """Normalize both parts and write the pair's yT2 columns.
                Both denominator rows are repacked to [128,16] with one
                SBUF->SBUF DMA each so the DVE reciprocal runs 128-wide,
                then sent back to a [2,1024] row pair feeding rank-1
                partition-broadcast matmuls."""
                drd = rdpool.tile([1, 2048], f32, tag="drd")
                nc.gpsimd.dma_start(out=drd[0:1, 0:1024], in_=ysb_c[64:65, :])
                nc.gpsimd.dma_start(out=drd[0:1, 1024:2048], in_=ysb_s[64:65, :])
                dd = rpool.tile([128, 16], f32, tag="dd")
                nc.gpsimd.dma_start(
                    out=dd[:].rearrange("p (b e) -> p b e", b=2),
                    in_=drd[:].rearrange("a (b p e) -> p (a b) e", b=2, p=128))
                rr = rpool.tile([128, 16], f16, tag="rr")
                with nc.allow_low_precision("softmax 1/denom in fp16"):
                    nc.vector.reciprocal(rr[:], dd[:])
                rrd = rdpool.tile([1, 2048], f16, tag="rrd")
                nc.gpsimd.dma_start(
                    out=rrd[:].rearrange("a (b p e) -> p (a b) e", b=2, p=128),
                    in_=rr[:].rearrange("p (b e) -> p b e", b=2))
                rrow = rpool.tile([1, 2048], f16, tag="rrow")
                nc.gpsimd.dma_start(out=rrow[:], in_=rrd[:])
                for half, dst_rows, shift in ((0, yT2_t[0:64, qlo:qlo + 512], False),
                                              (1, None, True)):
                    co, so_ = half * 512, half * 512 + 512
                    bcc = pjps.tile([64, 512], f32, tag="pj")
                    nc.tensor.matmul(bcc[:], ones_t[:], rrow[0:1, co:so_],
                                     start=True, stop=True)
                    yac = yapool.tile([64, 512], f32, tag="ya")
                    nc.vector.tensor_mul(yac[:], ysb_c[0:64, co:so_], bcc[:])
                    bcs = pjps.tile([64, 512], f32, tag="pj")
                    nc.tensor.matmul(bcs[:], ones_t[:],
                                     rrow[0:1, 1024 + co:1024 + so_],
                                     start=True, stop=True)
                    yas = yapool.tile([64, 512], f32, tag="ya")
                    nc.vector.tensor_mul(yas[:], ysb_s[0:64, co:so_], bcs[:])
                    if not shift:
                        nc.vector.tensor_add(dst_rows, yas[:], yac[:])
                    else:
                        ybsum = ybspool.tile([64, 512], f16, tag="ybsum")
                        nc.vector.tensor_add(ybsum[:], yas[:], yac[:])
                        # partition shift rows 0-63 -> 64-127 via SBUF-SBUF DMA
                        nc.gpsimd.dma_start(out=yT2_t[64:128, qlo:qlo + 512],
                                            in_=ybsum[:])

            def phase_c_pair(qlo):
                for tt in range(qlo // 128, qlo // 128 + 4):
                    so = sopool.tile([128, C], f16, tag="so")
                    for co in range(2):
                        po = pjps.tile([128, 512], f32, tag="pj")
                        nc.tensor.matmul(po[:],
                                         yT2_t[:, tt * 128:(tt + 1) * 128],
                                         wp_t[:, co * 512:(co + 1) * 512],
                                         start=True, stop=True)
                        if co == 0:
                            nc.vector.tensor_copy(
                                so[:, co * 512:(co + 1) * 512], po[:])
                        else:
                            nc.scalar.copy(
                                so[:, co * 512:(co + 1) * 512], po[:])
                    nc.gpsimd.dma_start(
                        out=out[tt * 128:(tt + 1) * 128, :], in_=so[:])

            # ---------------- fused pipeline ----------------
            proj_c_chunk(0)
            proj_x_chunk(0, split_load=True)
            proj_c_chunk(1)
            for i in range(1, NCH + 1):
                b, qc = divmod(i - 1, QC_PER_B)
                qlo = b * T + qc * 512
                part_c = attn_part(b, qc, qlo, is_self=False)
                part_s = attn_part(b, qc, qlo, is_self=True)
                if i < NCH:
                    proj_x_chunk(i)
                combine_pair(qlo, part_c, part_s)
                phase_c_pair(qlo)

    nc.compile()
    return nc


_NC_CACHE = {}


def _get_nc(zero_bias=False):
    if zero_bias not in _NC_CACHE:
        _NC_CACHE[zero_bias] = _build(zero_bias)
    return _NC_CACHE[zero_bias]


def warr(w):
    """[C,128] weight -> [128, KT, 128] fp16 (partition-major k-tiles)."""
    kt = w.shape[0] // 128
    return np.ascontiguousarray(
        w.reshape(kt, 128, w.shape[1]).transpose(1, 0, 2)).astype(np.float16)


def make_in_maps(x, cross_input, Wk, bk, Wq, bq, Wv, bv, Wck, bck, Wcq, bcq,
                 Wcv, bcv, Wp, bp):
    """Host-side shard + layout prep. Returns per-core input maps."""
    xT0 = np.asarray(x, np.float32).reshape(NT, C).T.astype(np.float16)  # [C, NT]
    xT = np.ascontiguousarray(
        xT0.reshape(KT_X, 128, NCH, 512).transpose(2, 1, 0, 3))  # [NCH,128,KT,512]
    cT0 = np.asarray(cross_input, np.float32).reshape(NTC, CC).T.astype(np.float16)
    cT = np.ascontiguousarray(
        cT0.reshape(KT_C, 128, NCHC, 512).transpose(2, 1, 0, 3))
    mask = np.triu(np.ones((128, 128), np.float32)).astype(np.float16)  # 1 iff kk<=qq
    Wq, Wk, Wv = (np.asarray(w, np.float32) for w in (Wq, Wk, Wv))
    Wcq, Wck, Wcv = (np.asarray(w, np.float32) for w in (Wcq, Wck, Wcv))
    Wp = np.asarray(Wp, np.float32)
    in_maps = []
    for c in range(NCORES):
        sl = slice(c * CPC, (c + 1) * CPC)
        bias6 = np.stack([np.asarray(v, np.float32)[sl] for v in
                          (bq, bk, bv, bcq, bck, bcv)], axis=1)
        in_maps.append({
            "xT": xT, "cT": cT,
            "wq": warr(Wq[:, sl]), "wk": warr(Wk[:, sl]),
            "wv": warr(Wv[:, sl]), "wcq": warr(Wcq[:, sl]),
            "wck": warr(Wck[:, sl]), "wcv": warr(Wcv[:, sl]),
            "wp": Wp[sl, :].astype(np.float16),
            "bias6": np.ascontiguousarray(bias6),
            "mask": mask,
            "ident": np.eye(128, dtype=np.float16),
        })
    return in_maps


def kernel(**inputs):
    in_maps = make_in_maps(**inputs)
    zb = all(not np.any(np.asarray(inputs[k])) for k in
             ("bq", "bk", "bv", "bcq", "bck", "bcv"))
    nc = _get_nc(zero_bias=zb)
    res = run_bass_kernel_spmd(nc, in_maps, list(range(NCORES)))
    acc = np.zeros((NT, C), np.float64)
    for c in range(NCORES):
        acc += res.results[c]["out"]
    acc += np.asarray(inputs["bp"], np.float32)
    return acc.reshape(B, T, C).astype(np.float32)


if __name__ == "__main__":
    nc = _get_nc()
    print("build + compile OK")


# revision 24
# speedup vs baseline: 1.3757x; 1.0646x over previous
"""Causal self-attention + cross-attention Trainium2 kernel (8 NeuronCores).

Sharding: head-parallel. 16 heads x 2 batches = 32 (b,h) pairs; core c owns
heads {2c, 2c+1} for both batches (its 128 channels of C=1024). Projections
are column-sliced per core; attention runs fully local per head; the output
projection is row-sliced and the 8 partial [B*T, C] outputs (fp16) are summed
on the host (no device collectives).

This version fuses the three phases into one software-pipelined stream:
iteration i projects x-chunk i while running attention for pair i-1 and the
output projection + store for pair i-2's tokens. PSUM is partitioned so the
phases can overlap: score tiles (2x2 banks), projection/output/transpose
tiles (2x1 banks), attention accumulators (2x1 banks). All PSUM evictions go
through the Vector engine (ScalarE is saturated by softmax exp). The softmax
epilogue is fully on-chip: the denominator row (accumulated by a ones-column
in V during the AV matmul) is reciprocal'd on DVE (fp16) and broadcast across
partitions with a rank-1 matmul instead of DMA round-trips.
"""
import sys

sys.path.insert(0, "/opt/trn_rl_repo")

import numpy as np

import concourse.bass as bass
import concourse.tile as tile
from concourse import bacc, mybir
from concourse.bass_utils import run_bass_kernel_spmd

dt = mybir.dt

B, T, TC, C, CC, H, D = 2, 2048, 512, 1024, 512, 16, 64
NCORES = 8
CPC = 128          # channels per core = 2 heads * 64
NT = B * T         # 4096 tokens (batch-major)
NTC = B * TC       # 1024 cross tokens
KT_X = C // 128    # 8 contraction tiles over C
KT_C = CC // 128   # 4 contraction tiles over CC
NCH = NT // 512    # 8 token chunks
NCHC = NTC // 512  # 2 cross token chunks
QC_PER_B = T // 512  # 4 q-chunks per batch
KT_PER_B = T // 128  # 16 k-tiles per batch
LOOKAHEAD = 3      # kt steps issued ahead of their AV in the PE queue


def _build(zero_bias=False):
    f32, f16 = dt.float32, dt.float16
    nc = bacc.Bacc("TRN2", target_bir_lowering=False, debug=False,
                   enable_asserts=True, num_devices=NCORES)

    xT = nc.dram_tensor("xT", [NCH, 128, KT_X, 512], f16, kind="ExternalInput").ap()
    cT = nc.dram_tensor("cT", [NCHC, 128, KT_C, 512], f16, kind="ExternalInput").ap()
    wq = nc.dram_tensor("wq", [128, KT_X, CPC], f16, kind="ExternalInput").ap()
    wk = nc.dram_tensor("wk", [128, KT_X, CPC], f16, kind="ExternalInput").ap()
    wv = nc.dram_tensor("wv", [128, KT_X, CPC], f16, kind="ExternalInput").ap()
    wcq = nc.dram_tensor("wcq", [128, KT_X, CPC], f16, kind="ExternalInput").ap()
    wck = nc.dram_tensor("wck", [128, KT_C, CPC], f16, kind="ExternalInput").ap()
    wcv = nc.dram_tensor("wcv", [128, KT_C, CPC], f16, kind="ExternalInput").ap()
    wp = nc.dram_tensor("wp", [CPC, C], f16, kind="ExternalInput").ap()
    bias6 = nc.dram_tensor("bias6", [CPC, 6], f32, kind="ExternalInput").ap()
    maskd = nc.dram_tensor("mask", [128, 128], f16, kind="ExternalInput").ap()
    identd = nc.dram_tensor("ident", [128, 128], f16, kind="ExternalInput").ap()
    out = nc.dram_tensor("out", [NT, C], f16, kind="ExternalOutput").ap()

    Exp = mybir.ActivationFunctionType.Exp
    SCALE = 0.125  # 1/sqrt(D)

    with tile.TileContext(nc) as tc:
        from contextlib import ExitStack
        with ExitStack() as es:
            persist = es.enter_context(tc.tile_pool(name="persist", bufs=1))
            qT_t = persist.tile([128, NT], f16, tag="qT")
            kT_t = persist.tile([128, NT], f16, tag="kT")
            qcT_t = persist.tile([128, NT], f16, tag="qcT")
            kcT_t = persist.tile([128, NTC], f16, tag="kcT")
            vT_t = persist.tile([128, NT], f16, tag="vT")
            vcT_t = persist.tile([128, NTC], f16, tag="vcT")
            vn_t = persist.tile([128, (NT // 128) * 130], f16, tag="vn")
            vcn_t = persist.tile([128, (NTC // 128) * 130], f16, tag="vcn")
            yT2_t = persist.tile([128, NT], f16, tag="yT2")
            wp_t = persist.tile([128, C], f16, tag="wp")
            bias_t = persist.tile([128, 6], f32, tag="bias")
            mask2_t = persist.tile([128, 256], f16, tag="mask2")
            ident_t = persist.tile([128, 128], f16, tag="ident")
            ones_t = persist.tile([1, 64], f16, tag="ones")

            wq_t = persist.tile([128, KT_X, CPC], f16, tag="wq")
            wk_t = persist.tile([128, KT_X, CPC], f16, tag="wk")
            wv_t = persist.tile([128, KT_X, CPC], f16, tag="wv")
            wcq_t = persist.tile([128, KT_X, CPC], f16, tag="wcq")
            wck_t = persist.tile([128, KT_C, CPC], f16, tag="wck")
            wcv_t = persist.tile([128, KT_C, CPC], f16, tag="wcv")

            # weight/constant loads; cross weights first (first compute),
            # spread across issue queues so dispatch doesn't serialize
            nc.sync.dma_start(out=wck_t[:], in_=wck[:])
            nc.scalar.dma_start(out=wcv_t[:], in_=wcv[:])
            nc.sync.dma_start(out=wq_t[:], in_=wq[:])
            nc.scalar.dma_start(out=wk_t[:], in_=wk[:])
            nc.sync.dma_start(out=wv_t[:], in_=wv[:])
            nc.scalar.dma_start(out=wcq_t[:], in_=wcq[:])
            nc.sync.dma_start(out=mask2_t[:, 0:128], in_=maskd[:])
            nc.sync.dma_start(out=mask2_t[:, 128:256], in_=maskd[:])
            nc.scalar.dma_start(out=ident_t[:], in_=identd[:])
            nc.sync.dma_start(out=bias_t[:], in_=bias6[:])
            nc.sync.dma_start(out=wp_t[:], in_=wp[:])
            nc.vector.memset(ones_t[:], 1.0)

            vn_r = vn_t[:].rearrange("p (t c) -> p t c", c=130)
            nc.vector.memset(vn_r[:, :, 64:65], 1.0)
            nc.vector.memset(vn_r[:, :, 129:130], 1.0)
            vcn_r = vcn_t[:].rearrange("p (t c) -> p t c", c=130)
            nc.vector.memset(vcn_r[:, :, 64:65], 1.0)
            nc.vector.memset(vcn_r[:, :, 129:130], 1.0)

            # working pools
            xpool = es.enter_context(tc.tile_pool(name="xpool", bufs=2))
            cpool = es.enter_context(tc.tile_pool(name="cpool", bufs=2))
            expool = es.enter_context(tc.tile_pool(name="expool", bufs=12))
            ysbpool = es.enter_context(tc.tile_pool(name="ysbpool", bufs=3))
            rpool = es.enter_context(tc.tile_pool(name="rpool", bufs=3))
            yapool = es.enter_context(tc.tile_pool(name="yapool", bufs=6))
            ybspool = es.enter_context(tc.tile_pool(name="ybspool", bufs=2))
            sopool = es.enter_context(tc.tile_pool(name="sopool", bufs=3))
            rdpool = es.enter_context(tc.tile_pool(name="rdpool", bufs=2,
                                                   space="DRAM"))
            # PSUM: st 2x2 banks, pj 2x1, yh0/yh1 1+1
            stps = es.enter_context(tc.tile_pool(name="stps", bufs=2, space="PSUM"))
            pjps = es.enter_context(tc.tile_pool(name="pjps", bufs=2, space="PSUM"))
            yh0ps = es.enter_context(tc.tile_pool(name="yh0ps", bufs=1, space="PSUM"))
            yh1ps = es.enter_context(tc.tile_pool(name="yh1ps", bufs=1, space="PSUM"))

            def psum_evict(dst_slice, ps, bcol):
                if zero_bias:
                    nc.vector.tensor_copy(dst_slice, ps[:])
                else:
                    nc.vector.tensor_scalar_add(dst_slice, ps[:],
                                                bias_t[:, bcol:bcol + 1])

            def transpose_v(src, dstn, tt):
                """PE-transpose one 128-token tile of v and scatter both
                heads' halves into the [64|1|64|1] AV layout in one copy."""
                pt = pjps.tile([128, 128], f16, tag="pj")
                nc.tensor.transpose(pt[:], src[:, tt * 128:(tt + 1) * 128],
                                    ident_t[:])
                src3 = pt[:].rearrange("p (b x) -> p b x", b=2)
                dst3 = dstn[:, tt * 130:(tt + 1) * 130].rearrange(
                    "p (b x) -> p b x", b=2, x=65)[:, :, 0:64]
                nc.vector.tensor_copy(dst3, src3)

            def proj_x_chunk(ch, split_load=False):
                xblk = xpool.tile([128, KT_X, 512], f16, tag="xblk")
                if split_load:
                    for kt in range(KT_X):
                        nc.gpsimd.dma_start(out=xblk[:, kt, :], in_=xT[ch, :, kt, :])
                else:
                    nc.gpsimd.dma_start(out=xblk[:], in_=xT[ch])
                for wtile, dst, bcol in ((wq_t, qT_t, 0), (wk_t, kT_t, 1),
                                         (wv_t, vT_t, 2), (wcq_t, qcT_t, 3)):
                    ps = pjps.tile([128, 512], f32, tag="pj")
                    for kt in range(KT_X):
                        nc.tensor.matmul(ps[:], wtile[:, kt, :], xblk[:, kt, :],
                                         start=(kt == 0), stop=(kt == KT_X - 1))
                    psum_evict(dst[:, ch * 512:(ch + 1) * 512], ps, bcol)
                    if dst is vT_t:
                        for tt in range(ch * 4, ch * 4 + 4):
                            transpose_v(vT_t, vn_t, tt)

            def proj_c_chunk(chc, split_load=False):
                cblk = cpool.tile([128, KT_C, 512], f16, tag="cblk")
                if split_load:
                    for kt in range(KT_C):
                        nc.gpsimd.dma_start(out=cblk[:, kt, :], in_=cT[chc, :, kt, :])
                else:
                    nc.gpsimd.dma_start(out=cblk[:], in_=cT[chc])
                for wtile, dst, bcol in ((wck_t, kcT_t, 4), (wcv_t, vcT_t, 5)):
                    ps = pjps.tile([128, 512], f32, tag="pj")
                    for kt in range(KT_C):
                        nc.tensor.matmul(ps[:], wtile[:, kt, :], cblk[:, kt, :],
                                         start=(kt == 0), stop=(kt == KT_C - 1))
                    psum_evict(dst[:, chc * 512:(chc + 1) * 512], ps, bcol)
                    if dst is vcT_t:
                        for tt in range(chc * 4, chc * 4 + 4):
                            transpose_v(vcT_t, vcn_t, tt)

            def attn_part(b, qc, qlo, is_self):
                """One softmax-attention accumulation (self or cross) for a
                512-wide q chunk of batch b. Returns the unnormalized
                [65,1024] SBUF accumulator (row 64 = denominators)."""
                nkt = (4 * qc + 4) if is_self else KT_C
                yh_0 = yh0ps.tile([65, 512], f32, tag="yh0")
                yh_1 = yh1ps.tile([65, 512], f32, tag="yh1")
                yh = (yh_0, yh_1)

                pend = []
                fidx = [0]

                def flush_one():
                    ex, off, vsrc, vc0, vc1 = pend.pop(0)
                    first = fidx[0] == 0
                    last = fidx[0] == nkt - 1
                    fidx[0] += 1
                    nc.tensor.matmul(
                        yh[0][:, off:512],
                        vsrc[:, vc0:vc0 + 65],
                        ex[:, off:512],
                        start=first, stop=last)
                    nc.tensor.matmul(
                        yh[1][:, off:512],
                        vsrc[:, vc1:vc1 + 65],
                        ex[:, 512 + off:1024],
                        start=first, stop=last)

                for kt in range(nkt):
                    if is_self:
                        crossing = kt >= 4 * qc
                        off = (kt - 4 * qc) * 128 if crossing else 0
                        klo = b * T + kt * 128
                        ksrc, qsrc, vsrc = kT_t, qT_t, vn_t
                        vbase = (b * KT_PER_B + kt) * 130
                    else:
                        crossing, off = False, 0
                        klo = b * TC + kt * 128
                        ksrc, qsrc, vsrc = kcT_t, qcT_t, vcn_t
                        vbase = (b * KT_C + kt) * 130
                    st = stps.tile([128, 1024], f32, tag="st")
                    nc.tensor.matmul(
                        st[:, off:512],
                        ksrc[0:64, klo:klo + 128],
                        qsrc[0:64, qlo + off:qlo + 512],
                        start=True, stop=True)
                    nc.tensor.matmul(
                        st[:, 512 + off:1024],
                        ksrc[64:128, klo:klo + 128],
                        qsrc[64:128, qlo + off:qlo + 512],
                        start=True, stop=True)
                    ex = expool.tile([128, 1024], f16, tag="ex")
                    if off == 0:
                        nc.scalar.activation(ex[:], st[:], Exp, scale=SCALE)
                    else:
                        st3 = st[:].rearrange("p (b x) -> p b x", b=2)
                        ex3 = ex[:].rearrange("p (b x) -> p b x", b=2)
                        nc.scalar.activation(ex3[:, :, off:512],
                                             st3[:, :, off:512],
                                             Exp, scale=SCALE)
                    if crossing:
                        exm = ex[:].rearrange("p (b x) -> p b x",
                                              b=2)[:, :, off:off + 128]
                        mk2 = mask2_t[:].rearrange("p (b x) -> p b x", b=2)
                        nc.vector.tensor_mul(exm, exm, mk2)
                    pend.append((ex, off, vsrc, vbase, vbase + 65))
                    if len(pend) > LOOKAHEAD:
                        flush_one()
                while pend:
                    flush_one()

                # evict accumulators to SBUF (releases the two yh banks),
                # and start the denominator row on its way to DRAM so the
                # repack in combine_pair has a short tail
                ysb = ysbpool.tile([65, 1024], f32, tag="ysb")
                nc.vector.tensor_copy(ysb[:, 0:512], yh[0][:])
                nc.vector.tensor_copy(ysb[:, 512:1024], yh[1][:])
                drd = rdpool.tile([1, 1024], f32, tag="drd")
                nc.sync.dma_start(out=drd[:], in_=ysb[64:65, :])
                return ysb, drd

            def combine_pair(qlo, part_c, part_s):
                """Normalize both parts and write the pair's yT2 columns.
                Both denominator rows are repacked to [128,16] so the DVE
                reciprocal runs 128-wide, then sent back to a [1,2048] row
                feeding rank-1 partition-broadcast matmuls."""
                ysb_c, drd_c = part_c
                ysb_s, drd_s = part_s
                dd = rpool.tile([128, 16], f32, tag="dd")
                nc.sync.dma_start(
                    out=dd[:, 0:8],
                    in_=drd_c[:].rearrange("a (p e) -> p (a e)", p=128))
                nc.sync.dma_start(
                    out=dd[:, 8:16],
                    in_=drd_s[:].rearrange("a (p e) -> p (a e)", p=128))
                rr = rpool.tile([128, 16], f16, tag="rr")
                with nc.allow_low_precision("softmax 1/denom in fp16"):
                    nc.vector.reciprocal(rr[:], dd[:])
                rrd = rdpool.tile([1, 2048], f16, tag="rrd")
                nc.sync.dma_start(
                    out=rrd[:].rearrange("a (b p e) -> p (a b) e", b=2, p=128),
                    in_=rr[:].rearrange("p (b e) -> p b e", b=2))
                rrow = rpool.tile([1, 2048], f16, tag="rrow")
                nc.sync.dma_start(out=rrow[:], in_=rrd[:])
                for half, dst_rows, shift in ((0, yT2_t[0:64, qlo:qlo + 512], False),
                                              (1, None, True)):
                    co, so_ = half * 512, half * 512 + 512
                    bcc = pjps.tile([64, 512], f32, tag="pj")
                    nc.tensor.matmul(bcc[:], ones_t[:], rrow[0:1, co:so_],
                                     start=True, stop=True)
                    yac = yapool.tile([64, 512], f32, tag="ya")
                    nc.vector.tensor_mul(yac[:], ysb_c[0:64, co:so_], bcc[:])
                    bcs = pjps.tile([64, 512], f32, tag="pj")
                    nc.tensor.matmul(bcs[:], ones_t[:],
                                     rrow[0:1, 1024 + co:1024 + so_],
                                     start=True, stop=True)
                    yas = yapool.tile([64, 512], f32, tag="ya")
                    nc.vector.tensor_mul(yas[:], ysb_s[0:64, co:so_], bcs[:])
                    if not shift:
                        nc.vector.tensor_add(dst_rows, yas[:], yac[:])
                    else:
                        ybsum = ybspool.tile([64, 512], f16, tag="ybsum")
                        nc.vector.tensor_add(ybsum[:], yas[:], yac[:])
                        # partition shift rows 0-63 -> 64-127 via SBUF-SBUF DMA
                        nc.sync.dma_start(out=yT2_t[64:128, qlo:qlo + 512],
                                          in_=ybsum[:])

            def phase_c_pair(qlo):
                for tt in range(qlo // 128, qlo // 128 + 4):
                    so = sopool.tile([128, C], f16, tag="so")
                    for co in range(2):
                        po = pjps.tile([128, 512], f32, tag="pj")
                        nc.tensor.matmul(po[:],
                                         yT2_t[:, tt * 128:(tt + 1) * 128],
                                         wp_t[:, co * 512:(co + 1) * 512],
                                         start=True, stop=True)
                        if co == 0:
                            nc.vector.tensor_copy(
                                so[:, co * 512:(co + 1) * 512], po[:])
                        else:
                            nc.scalar.copy(
                                so[:, co * 512:(co + 1) * 512], po[:])
                    eng = nc.gpsimd if tt % 2 == 0 else nc.sync
                    eng.dma_start(
                        out=out[tt * 128:(tt + 1) * 128, :], in_=so[:])

            # ---------------- fused pipeline ----------------
            proj_c_chunk(0, split_load=True)
            proj_x_chunk(0, split_load=True)
            proj_c_chunk(1)
            for i in range(1, NCH + 1):
                b, qc = divmod(i - 1, QC_PER_B)
                qlo = b * T + qc * 512
                part_c = attn_part(b, qc, qlo, is_self=False)
                part_s = attn_part(b, qc, qlo, is_self=True)
                if i < NCH:
                    proj_x_chunk(i)
                combine_pair(qlo, part_c, part_s)
                phase_c_pair(qlo)

    nc.compile()
    return nc


_NC_CACHE = {}


def _get_nc(zero_bias=False):
    if zero_bias not in _NC_CACHE:
        _NC_CACHE[zero_bias] = _build(zero_bias)
    return _NC_CACHE[zero_bias]


def warr(w):
    """[C,128] weight -> [128, KT, 128] fp16 (partition-major k-tiles)."""
    kt = w.shape[0] // 128
    return np.ascontiguousarray(
        w.reshape(kt, 128, w.shape[1]).transpose(1, 0, 2)).astype(np.float16)


def make_in_maps(x, cross_input, Wk, bk, Wq, bq, Wv, bv, Wck, bck, Wcq, bcq,
                 Wcv, bcv, Wp, bp):
    """Host-side shard + layout prep. Returns per-core input maps."""
    xT0 = np.asarray(x, np.float32).reshape(NT, C).T.astype(np.float16)  # [C, NT]
    xT = np.ascontiguousarray(
        xT0.reshape(KT_X, 128, NCH, 512).transpose(2, 1, 0, 3))  # [NCH,128,KT,512]
    cT0 = np.asarray(cross_input, np.float32).reshape(NTC, CC).T.astype(np.float16)
    cT = np.ascontiguousarray(
        cT0.reshape(KT_C, 128, NCHC, 512).transpose(2, 1, 0, 3))
    mask = np.triu(np.ones((128, 128), np.float32)).astype(np.float16)  # 1 iff kk<=qq
    Wq, Wk, Wv = (np.asarray(w, np.float32) for w in (Wq, Wk, Wv))
    Wcq, Wck, Wcv = (np.asarray(w, np.float32) for w in (Wcq, Wck, Wcv))
    Wp = np.asarray(Wp, np.float32)
    in_maps = []
    for c in range(NCORES):
        sl = slice(c * CPC, (c + 1) * CPC)
        bias6 = np.stack([np.asarray(v, np.float32)[sl] for v in
                          (bq, bk, bv, bcq, bck, bcv)], axis=1)
        in_maps.append({
            "xT": xT, "cT": cT,
            "wq": warr(Wq[:, sl]), "wk": warr(Wk[:, sl]),
            "wv": warr(Wv[:, sl]), "wcq": warr(Wcq[:, sl]),
            "wck": warr(Wck[:, sl]), "wcv": warr(Wcv[:, sl]),
            "wp": Wp[sl, :].astype(np.float16),
            "bias6": np.ascontiguousarray(bias6),
            "mask": mask,
            "ident": np.eye(128, dtype=np.float16),
        })
    return in_maps


def kernel(**inputs):
    in_maps = make_in_maps(**inputs)
    zb = all(not np.any(np.asarray(inputs[k])) for k in
             ("bq", "bk", "bv", "bcq", "bck", "bcv"))
    nc = _get_nc(zero_bias=zb)
    res = run_bass_kernel_spmd(nc, in_maps, list(range(NCORES)))
    acc = np.zeros((NT, C), np.float64)
    for c in range(NCORES):
        acc += res.results[c]["out"]
    acc += np.asarray(inputs["bp"], np.float32)
    return acc.reshape(B, T, C).astype(np.float32)


if __name__ == "__main__":
    nc = _get_nc()
    print("build + compile OK")
